# revision 8
# baseline (speedup 1.0000x reference)
"""Trainium2 Bass kernel for Luong 'general' attention scoring.

reference:
    proj     = einsum('sbh,kh->sbk', enc, W) + b          # [S,B,H]
    energies = einsum('bh,sbh->bs', hidden[0], proj)      # [B,S]
    out      = softmax(energies, -1)[:, None, :]          # [B,1,S]

Math reduction used here:
    energies[b,s] = hidden[b] . (W @ enc[s,b]) + hidden[b] . b_attn
                  = (W^T @ hidden[b]) . enc[s,b] + c_b
c_b is constant over s, so softmax is invariant to it -> b_attn drops out
entirely and the per-(s,b) work is a single H-length dot product against
q[b] = W^T @ hidden[b].  That turns the problem memory-bound: the cost is
streaming encoder_outputs (256 MB) once.

Sharding: data-parallel over batch. B=16 across 8 cores -> 2 batches/core.
Each core gets enc[:, 2i:2i+2, :] (32 MB, contiguous slice), the full W
(4 MB, replicated) and its hidden slice pre-transposed to [K,2] layout.

Per-core kernel:
  q[2,1024]   : 16 PE matmuls (W^T @ hidden), PSUM -> SBUF -> broadcast to
                q_bcast[128, 2048] via partition_broadcast (free = b*H + h)
  main loop   : 32 s-tiles. DMA enc tile [128, 2048] (1 MB, contiguous),
                one DVE tensor_mul tmp = enc_tile * q_bcast, then per b one
                ACT activation(Copy, accum_out) free-dim reduce:
                   e[s] = sum_h tmp[s, b*H:(b+1)*H]   -> et[128,2]
                (tensor_tensor_reduce would fuse these but crashes the exec
                unit on this HW/ucode combo — bisected via probe3.py)
  energies    : PE-transpose et [128,2] -> [2,128] PSUM, ACT-copy into
                energies[2, 4096]
  softmax     : DVE reduce_max -> ACT Exp(bias=-max, accum_out=Z) fused
                exp+sum -> DVE reciprocal -> DVE tensor_scalar mult
  out[2,4096] -> DMA
"""

import numpy as np

S = 4096
B = 16
H = 1024
N_CORES = 8
B_LOC = B // N_CORES          # 2
P = 128
NT = S // P                   # 32 s-tiles
KC = H // P                   # 8 k-chunks
FREE = B_LOC * H              # 2048

_cache = {}


def _build_nc():
    import concourse.bass as bass
    import concourse.tile as tile
    from concourse import bacc, mybir
    from concourse.masks import make_identity

    f32 = mybir.dt.float32
    # Bacc (not plain Bass): its compile() pass splits multi-sem waits on
    # matmuls (move_matmul_waits_to_ldweights / generate_event_semaphores),
    # without which walrus codegen rejects "Too many sync wait commands".
    nc = bacc.Bacc("TRN2")

    enc = nc.dram_tensor("enc", [S, FREE], f32, kind="ExternalInput")
    w = nc.dram_tensor("w", [H, H], f32, kind="ExternalInput")
    ht = nc.dram_tensor("ht", [P, KC * B_LOC], f32, kind="ExternalInput")
    out = nc.dram_tensor("out", [B_LOC, S], f32, kind="ExternalOutput")

    with tile.TileContext(nc) as tc:
        with (
            tc.tile_pool(name="singles", bufs=1) as singles,
            tc.tile_pool(name="encpool", bufs=8) as encpool,
            tc.tile_pool(name="tmppool", bufs=2) as tmppool,
            tc.tile_pool(name="tmp2pool", bufs=1) as tmp2pool,
            tc.tile_pool(name="etpool", bufs=4) as etpool,
            tc.tile_pool(name="qpsum", bufs=1, space="PSUM") as qpsum,
            tc.tile_pool(name="etpsum", bufs=4, space="PSUM") as etpsum,
            tc.tile_pool(name="dram", bufs=1, space="DRAM") as dram,
        ):
            ident = singles.tile([P, P], f32)
            make_identity(nc, ident)

            # hidden^T, laid out [k_part, (c, b)] so ht_sb[:, c, :] is the
            # [128, 2] stationary operand for k-chunk c.
            ht_sb = singles.tile([P, KC, B_LOC], f32)
            nc.sync.dma_start(out=ht_sb, in_=ht.rearrange("p (c b) -> p c b", b=B_LOC))

            # full W in one 4MB DMA: w_all[p, c, h] = W[c*128+p, h]
            w_all = singles.tile([P, KC, H], f32)
            nc.scalar.dma_start(
                out=w_all, in_=w.rearrange("(c p) h -> p c h", p=P)
            )

            # q[b, h] = sum_k hidden[b, k] * W[k, h]  -> PSUM [2, 1024]
            qp = qpsum.tile([B_LOC, H], f32)
            for c in range(KC):
                for ns in range(H // 512):
                    nc.tensor.matmul(
                        qp[:, ns * 512 : (ns + 1) * 512],
                        ht_sb[:, c, :],
                        w_all[:, c, ns * 512 : (ns + 1) * 512],
                        start=(c == 0),
                        stop=(c == KC - 1),
                    )
            q_sb = singles.tile([B_LOC, H], f32)
            nc.scalar.copy(out=q_sb, in_=qp)

            # broadcast q over all 128 partitions; free = b*H + h.
            # GPSIMD partition_broadcast needs a ucode library reload, so
            # instead roundtrip through a DRAM tile and re-load with a
            # partition-step-0 access pattern (128 partitions read the same
            # contiguous 8KB).
            q_dram = dram.tile([B_LOC, H], f32)
            nc.sync.dma_start(out=q_dram, in_=q_sb)
            q_bcast = singles.tile([P, FREE], f32)
            q_bcast_src = bass.AP(
                tensor=q_dram.tensor,
                offset=q_dram.offset,
                ap=[[0, P], [1, FREE]],
            )
            nc.sync.dma_start(out=q_bcast, in_=q_bcast_src)

            energies = singles.tile([B_LOC, S], f32)

            tmp2 = tmp2pool.tile([P, FREE], f32)
            for t in range(NT):
                enc_t = encpool.tile([P, FREE], f32, tag="enc")
                nc.sync.dma_start(out=enc_t, in_=enc[t * P : (t + 1) * P, :])
                tmp = tmppool.tile([P, FREE], f32, tag="tmp")
                nc.vector.tensor_mul(out=tmp, in0=enc_t, in1=q_bcast)
                et = etpool.tile([P, B_LOC], f32, tag="et")
                for b in range(B_LOC):
                    sl = slice(b * H, (b + 1) * H)
                    nc.scalar.activation(
                        out=tmp2[:, sl],
                        in_=tmp[:, sl],
                        func=mybir.ActivationFunctionType.Copy,
                        accum_out=et[:, b : b + 1],
                    )
                etp = etpsum.tile([B_LOC, P], f32, tag="etp")
                nc.tensor.transpose(etp, et, ident)
                nc.scalar.copy(out=energies[:, t * P : (t + 1) * P], in_=etp)

            # softmax over free dim (S) on 2 partitions
            mx = singles.tile([B_LOC, 1], f32)
            nc.vector.reduce_max(mx, energies, axis=mybir.AxisListType.X)
            negm = singles.tile([B_LOC, 1], f32)
            nc.vector.tensor_scalar_mul(out=negm, in0=mx, scalar1=-1.0)
            p_sb = singles.tile([B_LOC, S], f32)
            zsum = singles.tile([B_LOC, 1], f32)
            nc.scalar.activation(
                out=p_sb,
                in_=energies,
                func=mybir.ActivationFunctionType.Exp,
                bias=negm,
                scale=1.0,
                accum_out=zsum,
            )
            rz = singles.tile([B_LOC, 1], f32)
            nc.vector.reciprocal(rz, zsum)
            attn = singles.tile([B_LOC, S], f32)
            nc.vector.tensor_scalar_mul(out=attn, in0=p_sb, scalar1=rz)
            nc.sync.dma_start(out=out[:, :], in_=attn)

    nc.finalize()
    return nc


def get_nc():
    if "nc" not in _cache:
        _cache["nc"] = _build_nc()
    return _cache["nc"]


def make_in_maps(hidden, encoder_outputs, W_attn):
    """Shard full inputs into per-core input maps."""
    w_full = np.ascontiguousarray(W_attn, dtype=np.float32)
    in_maps = []
    for i in range(N_CORES):
        b0 = i * B_LOC
        enc_i = np.ascontiguousarray(
            encoder_outputs[:, b0 : b0 + B_LOC, :], dtype=np.float32
        ).reshape(S, FREE)
        # ht[p, c*B_LOC + b] = hidden[0, b0+b, c*128+p]
        h_i = np.ascontiguousarray(hidden[0, b0 : b0 + B_LOC, :], dtype=np.float32)
        ht_i = np.ascontiguousarray(
            h_i.reshape(B_LOC, KC, P).transpose(2, 1, 0).reshape(P, KC * B_LOC)
        )
        in_maps.append({"enc": enc_i, "w": w_full, "ht": ht_i})
    return in_maps


def kernel(hidden, encoder_outputs, W_attn, b_attn, **run_kwargs):
    """Full inputs in, full output out.  b_attn is mathematically irrelevant
    (constant shift per softmax row) and is ignored."""
    from concourse.bass_utils import run_bass_kernel_spmd

    nc = get_nc()
    in_maps = make_in_maps(hidden, encoder_outputs, W_attn)
    res = run_bass_kernel_spmd(
        nc, in_maps, core_ids=list(range(N_CORES)), **run_kwargs
    )
    out = np.empty((B, 1, S), dtype=np.float32)
    for i in range(N_CORES):
        out[i * B_LOC : (i + 1) * B_LOC, 0, :] = res.results[i]["out"]
    _cache["last_result"] = res
    return out


# revision 13
# speedup vs baseline: 1.1978x; 1.1978x over previous
"""Trainium2 Bass kernel for Luong 'general' attention scoring.

reference:
    proj     = einsum('sbh,kh->sbk', enc, W) + b          # [S,B,H]
    energies = einsum('bh,sbh->bs', hidden[0], proj)      # [B,S]
    out      = softmax(energies, -1)[:, None, :]          # [B,1,S]

Math reduction used here:
    energies[b,s] = hidden[b] . (W @ enc[s,b]) + hidden[b] . b_attn
                  = (W^T @ hidden[b]) . enc[s,b] + c_b
c_b is constant over s, so softmax is invariant to it -> b_attn drops out
entirely and the per-(s,b) work is a single H-length dot product against
q[b] = W^T @ hidden[b].  That turns the problem memory-bound: the cost is
streaming encoder_outputs (256 MB) once.

Sharding: data-parallel over batch. B=16 across 8 cores -> 2 batches/core.
Each core gets enc[:, 2i:2i+2, :] (32 MB, contiguous slice), the full W
(4 MB, replicated) and its hidden slice pre-transposed to [K,2] layout.

Per-core kernel (v2, engine-balanced):
  prologue    : W chunks head the sync DMA ring (full bandwidth, ~11us),
                16 PE matmuls compute q[2,1024] chunk-by-chunk as W lands,
                then q broadcast to q_bcast[128, 2048] via 4 PE matmuls with
                one-hot-row [2,128] lhsT (no DRAM roundtrip).
  main loop   : 32 s-tiles. enc tile [128, 2048] (1 MB contiguous DMA),
                DVE tensor_mul tmp = enc_tile * q_bcast (~2.3us), per b an
                ACT activation(Copy, accum_out) free-dim reduce (~1.4us),
                PE-transpose et[128,2] straight into the [2,4096] PSUM
                energies tile (no per-tile PSUM->SBUF copy).
                (tensor_tensor_reduce would fuse mul+reduce on DVE but
                crashes the exec unit on this HW/ucode combo; GpSimd offload
                loses to the DVE<->GpSimd shared-SBUF-port exclusive lock.)
  softmax     : partial maxes every 8 tiles on DVE (overlapped), single ACT
                Exp(bias=-max, accum_out=Z) pass, DVE reciprocal + in-place
                scale, DMA out.
"""

import numpy as np

S = 4096
B = 16
H = 1024
N_CORES = 8
B_LOC = B // N_CORES          # 2
P = 128
NT = S // P                   # 32 s-tiles
KC = H // P                   # 8 k-chunks
FREE = B_LOC * H              # 2048

_cache = {}


def _build_nc():
    import concourse.bass as bass
    import concourse.tile as tile
    from concourse import bacc, mybir
    from concourse.masks import make_identity

    f32 = mybir.dt.float32
    # Bacc (not plain Bass): its compile() pass splits multi-sem waits on
    # matmuls (move_matmul_waits_to_ldweights / generate_event_semaphores),
    # without which walrus codegen rejects "Too many sync wait commands".
    nc = bacc.Bacc("TRN2")

    enc = nc.dram_tensor("enc", [S, FREE], f32, kind="ExternalInput")
    w = nc.dram_tensor("w", [H, H], f32, kind="ExternalInput")
    ht = nc.dram_tensor("ht", [P, KC * B_LOC], f32, kind="ExternalInput")
    sel = nc.dram_tensor("sel", [B_LOC, B_LOC * P], f32, kind="ExternalInput")
    out = nc.dram_tensor("out", [B_LOC, S], f32, kind="ExternalOutput")

    with tile.TileContext(nc) as tc:
        with (
            tc.tile_pool(name="singles", bufs=1) as singles,
            tc.tile_pool(name="encpool", bufs=8) as encpool,
            tc.tile_pool(name="tmppool", bufs=2) as tmppool,
            tc.tile_pool(name="tmp2pool", bufs=1) as tmp2pool,
            tc.tile_pool(name="etpool", bufs=4) as etpool,
        ):
            ident = singles.tile([P, P], f32)
            make_identity(nc, ident)

            # one-hot row selectors for the q broadcast matmuls (host const;
            # memset at partition offset 1 trips the start-partition check)
            onehot = singles.tile([B_LOC, B_LOC, P], f32)
            nc.scalar.dma_start(
                out=onehot, in_=sel.rearrange("p (b m) -> p b m", b=B_LOC)
            )

            # hidden^T on the scalar ring (parallel with W on sync ring)
            ht_sb = singles.tile([P, KC, B_LOC], f32)
            nc.scalar.dma_start(
                out=ht_sb, in_=ht.rearrange("p (c b) -> p c b", b=B_LOC)
            )

            # W chunks at the HEAD of the sync ring: full HBM bandwidth for
            # the q-prep critical path before enc streaming begins.
            w_all = singles.tile([P, KC, H], f32)
            for c in range(KC):
                nc.sync.dma_start(
                    out=w_all[:, c, :], in_=w[c * P : (c + 1) * P, :]
                )

            q_sb = singles.tile([B_LOC, H], f32)
            q_bcast = singles.tile([P, FREE], f32)
            pmax = singles.tile([B_LOC, NT // 8], f32)
            p_sb = singles.tile([B_LOC, S], f32)

            with tc.tile_pool(name="psA", bufs=1, space="PSUM") as psA:
                # q[b, h] = sum_k hidden[b, k] * W[k, h]  -> PSUM [2, 1024]
                qp = psA.tile([B_LOC, H], f32)
                for c in range(KC):
                    for ns in range(H // 512):
                        nc.tensor.matmul(
                            qp[:, ns * 512 : (ns + 1) * 512],
                            ht_sb[:, c, :],
                            w_all[:, c, ns * 512 : (ns + 1) * 512],
                            start=(c == 0),
                            stop=(c == KC - 1),
                        )
                nc.scalar.copy(out=q_sb, in_=qp)

                # broadcast q over 128 partitions: qb_b = onehot_b.T @ q_sb
                for b in range(B_LOC):
                    qb = psA.tile([P, H], f32, tag="qb")
                    for ns in range(H // 512):
                        nc.tensor.matmul(
                            qb[:, ns * 512 : (ns + 1) * 512],
                            onehot[:, b, :],
                            q_sb[:, ns * 512 : (ns + 1) * 512],
                            start=True,
                            stop=True,
                        )
                    nc.scalar.copy(out=q_bcast[:, b * H : (b + 1) * H], in_=qb)

            with tc.tile_pool(name="psB", bufs=1, space="PSUM") as psB:
                # all 32 transposed energy columns land directly in PSUM
                energies = psB.tile([B_LOC, S], f32)

                tmp2 = tmp2pool.tile([P, FREE], f32)
                for t in range(NT):
                    enc_t = encpool.tile([P, FREE], f32, tag="enc")
                    nc.sync.dma_start(out=enc_t, in_=enc[t * P : (t + 1) * P, :])
                    tmp = tmppool.tile([P, FREE], f32, tag="tmp")
                    nc.vector.tensor_mul(out=tmp, in0=enc_t, in1=q_bcast)
                    et = etpool.tile([P, B_LOC], f32, tag="et")
                    for b in range(B_LOC):
                        sl = slice(b * H, (b + 1) * H)
                        nc.scalar.activation(
                            out=tmp2[:, sl],
                            in_=tmp[:, sl],
                            func=mybir.ActivationFunctionType.Copy,
                            accum_out=et[:, b : b + 1],
                        )
                    nc.tensor.transpose(
                        energies[:, t * P : (t + 1) * P], et, ident
                    )
                    if t % 8 == 7:
                        k = t // 8
                        nc.vector.reduce_max(
                            pmax[:, k : k + 1],
                            energies[:, k * 1024 : (k + 1) * 1024],
                            axis=mybir.AxisListType.X,
                        )

                # softmax over free dim (S) on 2 partitions
                mx = singles.tile([B_LOC, 1], f32)
                nc.vector.reduce_max(mx, pmax, axis=mybir.AxisListType.X)
                negm = singles.tile([B_LOC, 1], f32)
                nc.vector.tensor_scalar_mul(out=negm, in0=mx, scalar1=-1.0)
                zsum = singles.tile([B_LOC, 1], f32)
                nc.scalar.activation(
                    out=p_sb,
                    in_=energies,
                    func=mybir.ActivationFunctionType.Exp,
                    bias=negm,
                    scale=1.0,
                    accum_out=zsum,
                )
            rz = singles.tile([B_LOC, 1], f32)
            nc.vector.reciprocal(rz, zsum)
            nc.vector.tensor_scalar_mul(out=p_sb, in0=p_sb, scalar1=rz)
            nc.sync.dma_start(out=out[:, :], in_=p_sb)

    nc.finalize()
    return nc


def get_nc():
    if "nc" not in _cache:
        _cache["nc"] = _build_nc()
    return _cache["nc"]


def make_in_maps(hidden, encoder_outputs, W_attn):
    """Shard full inputs into per-core input maps."""
    w_full = np.ascontiguousarray(W_attn, dtype=np.float32)
    in_maps = []
    for i in range(N_CORES):
        b0 = i * B_LOC
        enc_i = np.ascontiguousarray(
            encoder_outputs[:, b0 : b0 + B_LOC, :], dtype=np.float32
        ).reshape(S, FREE)
        # ht[p, c*B_LOC + b] = hidden[0, b0+b, c*128+p]
        h_i = np.ascontiguousarray(hidden[0, b0 : b0 + B_LOC, :], dtype=np.float32)
        ht_i = np.ascontiguousarray(
            h_i.reshape(B_LOC, KC, P).transpose(2, 1, 0).reshape(P, KC * B_LOC)
        )
        sel = np.zeros((B_LOC, B_LOC, P), dtype=np.float32)
        for b in range(B_LOC):
            sel[b, b, :] = 1.0
        sel = sel.reshape(B_LOC, B_LOC * P)
        in_maps.append({"enc": enc_i, "w": w_full, "ht": ht_i, "sel": sel})
    return in_maps


def kernel(hidden, encoder_outputs, W_attn, b_attn, **run_kwargs):
    """Full inputs in, full output out.  b_attn is mathematically irrelevant
    (constant shift per softmax row) and is ignored."""
    from concourse.bass_utils import run_bass_kernel_spmd

    nc = get_nc()
    in_maps = make_in_maps(hidden, encoder_outputs, W_attn)
    res = run_bass_kernel_spmd(
        nc, in_maps, core_ids=list(range(N_CORES)), **run_kwargs
    )
    out = np.empty((B, 1, S), dtype=np.float32)
    for i in range(N_CORES):
        out[i * B_LOC : (i + 1) * B_LOC, 0, :] = res.results[i]["out"]
    _cache["last_result"] = res
    return out
